# revision 14
# baseline (speedup 1.0000x reference)
"""Trainium2 Bass kernel for the CPN/WCP loss (ce + Sinkhorn wcp).

Strategy (v3):
  - Host ships features.T in bf16 ("featT") -> no on-chip F transposes;
    fp32 row slab "fslice" for 4 tiny fsT transposes; bf16 target mask.
  - ph slab [64,512] = fs@F^T - 0.5*sq_j in ONE PSUM bank: 4 bf16 dot
    matmuls + ONE bf16 (-0.5)*colsum matmul against the pre-summed
    FTsqS = sum_q FT[q]^2 (3 DVE adds replace 3 PE matmuls).
  - p1 softmax bias-free; CE via per-window max (mh) + masked target
    column (dcol): ce = lnS54 + S5*(mh-dcol). CE work is emitted into
    the Sinkhorn loop's DVE bubbles (it only feeds the output).
  - cost matrix from gT = sum of FT column blocks: G0 = gT^T gT (PSUM),
    rn = rsqrt(diag); per-row min/max normalization is invariant to the
    per-row scale so the normalized Gram maps exactly onto reference.
  - Multiplicative Sinkhorn, 2 half-chains of 128 problems:
    a = p1*recip(KT@b), b = recip(K2@a), K2 = 128K (folds p2=1/128).
    Iter-1 fold: pz1 = (K2*recip(K2@1))^T @ p1T -- row-scaled K2s; the
    resulting 1/128 scale on a and 128 on b cancels in the wcp bilinear
    form. Converged to <1e-5 by iter 1; we run 3 iterations.
  - wcp: pw = KC@a_last; TENSOR_TENSOR_REDUCE fuses pw*b + row-reduce.
"""

import sys

for _p in ("/opt/trn_rl_repo",):
    if _p not in sys.path:
        sys.path.insert(0, _p)

import numpy as np

AUG = 4
B = 128
D = 512
N = AUG * B          # 512 feature rows
NCORES = 8
RPC = N // NCORES    # 64 slab rows per core
MPC = RPC * AUG      # 256 sinkhorn problems per core
HB = MPC // 2        # 128 problems per half-chain
M_TOT = N * AUG      # 2048
TEMP = 5.0
GAMMA = 0.2
SINK_ITR = 3         # converged to <1e-5 of reference by iter 1
SCALE1 = 2.0 / float(np.sqrt(np.float32(D)))
SCALE5 = 2.0 / TEMP

_CACHE = {}


def _build_nc(stage=99):
    import concourse.bacc as bacc
    import concourse.tile as tile
    import concourse.mybir as mybir
    from concourse.dve_ops import (RECIP_APPROX_FAST_CONSTS as _RAFC,
                                   RECIPROCAL_APPROX_FAST as _RAF,
                                   TENSOR_TENSOR_REDUCE as _TTR)

    dt = mybir.dt.float32
    dtb = mybir.dt.bfloat16
    fp = mybir.ActivationFunctionType
    alu = mybir.AluOpType
    ax = mybir.AxisListType

    nc = bacc.Bacc(
        "TRN2",
        target_bir_lowering=False,
        debug=False,
        enable_asserts=False,
        num_devices=NCORES,
    )

    featT = nc.dram_tensor("featT", [D, N], dtb, kind="ExternalInput").ap()
    fsl = nc.dram_tensor("fslice", [RPC, D], dt, kind="ExternalInput").ap()
    mce = nc.dram_tensor("maskce", [RPC, B], dtb, kind="ExternalInput").ap()
    outd = nc.dram_tensor("out", [1, 256], dt, kind="ExternalOutput").ap()

    with tile.TileContext(nc) as tc:
        with (
            tc.tile_pool(name="sb", bufs=1) as sb,
            tc.tile_pool(name="scr", bufs=2) as scr,
            tc.tile_pool(name="ps_big", bufs=4, space="PSUM") as psb,
            tc.tile_pool(name="ps_t", bufs=2, space="PSUM") as pst,
            tc.tile_pool(name="ps_h", bufs=1, space="PSUM") as psh,
        ):
            dbg = None

            _tabs = list(__import__("concourse.hw_specs",
                                    fromlist=["hw_specs"]
                                    ).get_activation_tables(nc.m.arch))
            _set_lnexp = _tabs.index("natural_log_exp_and_others")
            nc.scalar.add_instruction(mybir.InstLoadActFuncSet(
                name=nc.get_next_instruction_name(), ins=[], outs=[],
                act_func_set_id=_set_lnexp))

            # ---------------- constants that gate PE ----------------
            ones_f = sb.tile([128, 128], dt, tag="ones_f", name="ones_f")
            nc.vector.memset(ones_f[:], 1.0)
            I_f = sb.tile([128, 128], dt, tag="I_f", name="I_f")
            nc.gpsimd.affine_select(I_f[:], ones_f[:], [[1, 128]],
                                    alu.is_equal, 0.0, base=0,
                                    channel_multiplier=-1)

            # ---------------- DMAs (3 issuing queues) ----------------
            fs = sb.tile([RPC, D], dt, tag="fs", name="fs")
            nc.sync.dma_start(out=fs[:], in_=fsl[:])
            FT = [sb.tile([128, N], dtb, tag=f"FT{q}", name=f"FT{q}")
                  for q in range(4)]
            # (q, half, engine): FT0 lands first, FT3 last
            for q, h, eng in ((0, 0, nc.sync), (0, 1, nc.gpsimd),
                              (1, 0, nc.scalar), (1, 1, nc.sync),
                              (2, 0, nc.gpsimd), (2, 1, nc.scalar),
                              (3, 0, nc.sync), (3, 1, nc.gpsimd)):
                eng.dma_start(
                    out=FT[q][h * 64:(h + 1) * 64, :],
                    in_=featT[q * 128 + h * 64:q * 128 + (h + 1) * 64, :])
            mk = sb.tile([RPC, B], dtb, tag="mk", name="mk")
            nc.scalar.dma_start(out=mk[:], in_=mce[:])

            # ---------------- remaining constants ----------------
            ones_b = sb.tile([128, 128], dtb, tag="ones_b", name="ones_b")
            nc.vector.memset(ones_b[:], 1.0)
            I_b = sb.tile([128, 128], dtb, tag="I_b", name="I_b")
            nc.gpsimd.affine_select(I_b[:], ones_b[:], [[1, 128]],
                                    alu.is_equal, 0.0, base=0,
                                    channel_multiplier=-1)
            negq = sb.tile([128, RPC], dtb, tag="negq", name="negq")
            nc.gpsimd.memset(negq[:], -0.5)
            ln128t = sb.tile([128, 1], dt, tag="ln128t", name="ln128t")
            nc.gpsimd.memset(ln128t[:], float(np.log(128.0)))
            outS = sb.tile([1, 256], dt, tag="outS", name="outS")
            nc.vector.memset(outS[:], 0.0)

            ce_part = None
            wcp_part = None

            if stage >= 1:
                # fsT via PE transposes (fs + I_f land first)
                fsT = []
                for q in range(4):
                    pt = pst.tile([128, RPC], dt, tag="pt", name=f"ptf{q}")
                    nc.tensor.transpose(pt[:], fs[:, q * 128:(q + 1) * 128],
                                        I_f[:RPC, :RPC])
                    fq = sb.tile([128, RPC], dtb, tag=f"fsT{q}",
                                 name=f"fsT{q}")
                    nc.vector.tensor_copy(fq[:], pt[:])
                    fsT.append(fq)

                # FT^2 on ACT, then pre-sum the 4 squared tiles on DVE
                FTsq = []
                for q in range(4):
                    sq = scr.tile([128, N], dtb, tag=f"FTsq{q & 1}",
                                  name=f"FTsq{q}")
                    nc.scalar.activation(sq[:], FT[q][:], fp.Square)
                    FTsq.append(sq)
                s01 = scr.tile([128, N], dtb, tag="s01", name="s01")
                nc.vector.tensor_add(s01[:], FTsq[0][:], FTsq[1][:])
                s23 = scr.tile([128, N], dtb, tag="s23", name="s23")
                nc.vector.tensor_add(s23[:], FTsq[2][:], FTsq[3][:])
                sqS = sb.tile([128, N], dtb, tag="sqS", name="sqS")
                nc.vector.tensor_add(sqS[:], s01[:], s23[:])

                # gT[q] = sum of the 4 column blocks of FT[q]
                gT = []
                for q in range(4):
                    t1 = scr.tile([128, 128], dtb, tag="gt1", name=f"gt1{q}")
                    nc.vector.tensor_add(t1[:], FT[q][:, 0:128],
                                         FT[q][:, 128:256])
                    t2 = scr.tile([128, 128], dtb, tag="gt2", name=f"gt2{q}")
                    nc.gpsimd.tensor_add(t2[:], FT[q][:, 256:384],
                                         FT[q][:, 384:512])
                    gq = sb.tile([128, 128], dtb, tag=f"gT{q}", name=f"gT{q}")
                    nc.vector.tensor_add(gq[:], t1[:], t2[:])
                    gT.append(gq)
                dbg = fsT[0]

            if stage >= 2:
                # ph = fs@F^T - 0.5*sq_j ; G0 = gT^T gT  (interleaved on PE)
                ph = psh.tile([RPC, D], dt, tag="ph", name="ph")
                G0 = psb.tile([128, 128], dt, tag="big", name="G0")
                for q in range(4):
                    nc.tensor.matmul(ph[:], fsT[q][:], FT[q][:],
                                     start=(q == 0), stop=False)
                    nc.tensor.matmul(G0[:], gT[q][:], gT[q][:],
                                     start=(q == 0), stop=(q == 3))
                nc.tensor.matmul(ph[:], negq[:], sqS[:],
                                 start=False, stop=True)
                if stage == 2:
                    dbg = sb.tile([RPC, 1], dt, tag="dbg2", name="dbg2")
                    nc.vector.tensor_copy(dbg[:], ph[:, 0:1])

            if stage >= 3:
                # ---- cost branch: diag -> rsqrt -> normalized Gram ----
                dgm = scr.tile([128, 128], dt, tag="dgm", name="dgm")
                nc.vector.tensor_mul(dgm[:], G0[:], I_f[:])
                dg = sb.tile([128, 1], dt, tag="dg", name="dg")
                nc.vector.tensor_reduce(dg[:], dgm[:], axis=ax.X, op=alu.add)
                lndg = sb.tile([128, 1], dt, tag="lndg", name="lndg")
                nc.scalar.activation(lndg[:], dg[:], fp.Ln)
                rn = sb.tile([128, 1], dt, tag="rn", name="rn")
                nc.scalar.activation(rn[:], lndg[:], fp.Exp, scale=-0.5)
                rnb = sb.tile([128, 1], dtb, tag="rnb", name="rnb")
                nc.gpsimd.tensor_copy(rnb[:], rn[:])
                rnB = psb.tile([128, 128], dt, tag="big", name="rnB")
                nc.tensor.matmul(rnB[:], rnb[:, 0:1].to_broadcast((128, 128)),
                                 I_b[:], start=True, stop=True)
                z = sb.tile([128, 128], dt, tag="z", name="z")
                nc.vector.tensor_scalar_mul(z[:], G0[:], rn[:, 0:1])
                y = sb.tile([128, 128], dt, tag="y", name="y")
                nc.vector.tensor_mul(y[:], z[:], rnB[:])
                ymax = sb.tile([128, 1], dt, tag="ymax", name="ymax")
                nc.vector.tensor_reduce(ymax[:], y[:], axis=ax.X, op=alu.max)
                ymin = sb.tile([128, 1], dt, tag="ymin", name="ymin")
                nc.vector.tensor_reduce(ymin[:], y[:], axis=ax.X, op=alu.min)
                den = sb.tile([128, 1], dt, tag="den", name="den")
                nc.gpsimd.tensor_sub(den[:], ymax[:], ymin[:])
                rden = sb.tile([128, 1], dt, tag="rden", name="rden")
                nc.vector.reciprocal(rden[:], den[:])
                # costm = (ymax - y)*GAMMA*rden + I
                #       = y*(-GAMMA*rden) + (ymax*GAMMA*rden) + I
                sG = sb.tile([128, 1], dt, tag="sG", name="sG")
                nc.gpsimd.tensor_scalar_mul(sG[:], rden[:], -GAMMA)
                sGm = sb.tile([128, 1], dt, tag="sGm", name="sGm")
                nc.gpsimd.tensor_mul(sGm[:], ymax[:], sG[:])
                cma = scr.tile([128, 128], dt, tag="cma", name="cma")
                nc.vector.tensor_scalar(
                    out=cma[:], in0=y[:], scalar1=sG[:, 0:1],
                    scalar2=sGm[:, 0:1], op0=alu.mult, op1=alu.subtract)
                costm = sb.tile([128, 128], dt, tag="costm", name="costm")
                nc.gpsimd.tensor_add(costm[:], cma[:], I_f[:])

                # ---- p1 path (row layout) ----
                E1 = sb.tile([RPC, D], dt, tag="E1", name="E1")
                nc.scalar.activation(E1[:], ph[:], fp.Exp, scale=SCALE1)
                S14 = sb.tile([RPC, 4], dt, tag="S14", name="S14")
                nc.vector.tensor_reduce(
                    S14[:], E1[:].rearrange("p (k x) -> p k x", k=4),
                    axis=ax.X, op=alu.add)
                rS14 = sb.tile([RPC, 4], dt, tag="rS14", name="rS14")
                nc.vector.reciprocal(rS14[:], S14[:])
                p1r = sb.tile([RPC, D], dt, tag="p1r", name="p1r")
                for k in range(4):
                    ksl = slice(k * 128, (k + 1) * 128)
                    nc.vector.tensor_scalar(
                        out=p1r[:, ksl], in0=E1[:, ksl],
                        scalar1=rS14[:, k:k + 1], scalar2=1e-12,
                        op0=alu.mult, op1=alu.add)

                # ---- K tiles (K2 first: it gates the loop entry) ----
                K2 = sb.tile([128, 128], dtb, tag="K2", name="K2")
                nc.scalar.activation(K2[:], costm[:], fp.Exp,
                                     bias=ln128t[:, 0:1], scale=-2.0)
                ptK = pst.tile([128, 128], dt, tag="pt", name="ptK")
                nc.tensor.transpose(ptK[:], costm[:], I_f[:])
                KT = sb.tile([128, 128], dtb, tag="KT", name="KT")
                nc.scalar.activation(KT[:], ptK[:], fp.Exp, scale=-2.0)
                K = sb.tile([128, 128], dt, tag="K", name="K")
                nc.scalar.activation(K[:], costm[:], fp.Exp, scale=-2.0)
                KC = sb.tile([128, 128], dtb, tag="KC", name="KC")
                nc.gpsimd.tensor_mul(KC[:], K[:], costm[:])
                # K2 row sums (free-axis reduce); fold recip into K2s rows
                # (the 128x scale on b / 1/128 on a cancels in the wcp
                # bilinear form)
                rsum = sb.tile([128, 1], dt, tag="rsum", name="rsum")
                nc.vector.tensor_reduce(rsum[:], K2[:], axis=ax.X, op=alu.add)
                rs0 = sb.tile([128, 1], dt, tag="rs0", name="rs0")
                nc.vector.reciprocal(rs0[:], rsum[:])
                K2s = sb.tile([128, 128], dtb, tag="K2s", name="K2s")
                nc.vector.tensor_scalar_mul(K2s[:], K2[:], rs0[:, 0:1])
                if stage == 3:
                    dbg = sb.tile([128, 1], dt, tag="dbg3", name="dbg3")
                    nc.vector.tensor_copy(dbg[:], K[:, 0:1])

            if stage >= 4:
                # ---- p1T transposes ----
                p1T = sb.tile([128, MPC], dtb, tag="p1T", name="p1T")
                for k in range(4):
                    pt = pst.tile([128, RPC], dt, tag="pt", name=f"ptp{k}")
                    nc.tensor.transpose(pt[:], p1r[:, k * 128:(k + 1) * 128],
                                        I_f[:RPC, :RPC])
                    nc.scalar.copy(p1T[:, k * RPC:(k + 1) * RPC], pt[:])
                if stage == 4:
                    dbg = sb.tile([128, 1], dt, tag="dbg4", name="dbg4")
                    nc.vector.tensor_copy(dbg[:], p1T[:, 0:1])

            # CE vector-engine ops are emitted interleaved into the loop
            # below (they only feed the output, and DVE has loop bubbles).
            ce_box = []

            def _ce_ops():
                mh = sb.tile([RPC, 4], dt, tag="mh", name="mh")
                nc.vector.tensor_reduce(
                    mh[:], ph[:].rearrange("p (k x) -> p k x", k=4),
                    axis=ax.X, op=alu.max)
                yield
                dcm = scr.tile([RPC, D], dt, tag="dcm", name="dcm")
                nc.vector.tensor_mul(dcm[:, 0:128], ph[:, 0:128], mk[:])
                nc.vector.tensor_mul(dcm[:, 128:256], ph[:, 128:256], mk[:])
                yield
                bias5 = sb.tile([RPC, 4], dt, tag="bias5", name="bias5")
                nc.gpsimd.tensor_scalar_mul(bias5[:], mh[:], -SCALE5)
                E2 = scr.tile([RPC, D], dt, tag="E2", name="E2")
                for k in range(4):
                    ksl = slice(k * 128, (k + 1) * 128)
                    nc.scalar.activation(E2[:, ksl], ph[:, ksl], fp.Exp,
                                         bias=bias5[:, k:k + 1], scale=SCALE5)
                nc.vector.tensor_mul(dcm[:, 256:384], ph[:, 256:384], mk[:])
                nc.vector.tensor_mul(dcm[:, 384:512], ph[:, 384:512], mk[:])
                yield
                dcol4 = sb.tile([RPC, 4], dt, tag="dcol4", name="dcol4")
                nc.vector.tensor_reduce(
                    dcol4[:], dcm[:].rearrange("p (k x) -> p k x", k=4),
                    axis=ax.X, op=alu.add)
                yield
                S54 = sb.tile([RPC, 4], dt, tag="S54", name="S54")
                nc.vector.tensor_reduce(
                    S54[:], E2[:].rearrange("p (k x) -> p k x", k=4),
                    axis=ax.X, op=alu.add)
                lnS54 = sb.tile([RPC, 4], dt, tag="lnS54", name="lnS54")
                nc.scalar.activation(lnS54[:], S54[:], fp.Ln)
                ce4a = sb.tile([RPC, 4], dt, tag="ce4a", name="ce4a")
                nc.gpsimd.tensor_sub(ce4a[:], mh[:], dcol4[:])
                yield
                ce4 = sb.tile([RPC, 4], dt, tag="ce4", name="ce4")
                nc.vector.scalar_tensor_tensor(
                    out=ce4[:], in0=ce4a[:], scalar=SCALE5,
                    in1=lnS54[:], op0=alu.mult, op1=alu.add)
                cep = sb.tile([RPC, 1], dt, tag="ce_part", name="ce_part")
                nc.vector.tensor_reduce(cep[:], ce4[:], axis=ax.X,
                                        op=alu.add)
                ce_box.append(cep)
                yield

            if stage >= 5:
                ce_gen = _ce_ops()

                def _ce_step():
                    try:
                        next(ce_gen)
                    except StopIteration:
                        pass

                # ---- Sinkhorn loop ----
                _c = _RAFC
                As = [None, None]
                bs = [None, None]
                pws = [None, None]
                # iter 1 (folded): pz1 = K2s^T @ p1T
                pzs = []
                for h in range(2):
                    pz = psb.tile([128, HB], dt, tag="big", name=f"pz1{h}")
                    nc.tensor.matmul(pz[:], K2s[:],
                                     p1T[:, h * HB:(h + 1) * HB],
                                     start=True, stop=True)
                    pzs.append(pz)
                for h in range(2):
                    bh = scr.tile([128, HB], dtb, tag=f"b{h}", name=f"b1{h}")
                    nc.vector._custom_dve(_RAF, out=bh[:], in0=pzs[h][:],
                                          s0=_c["s0"], s1=_c["s1"],
                                          imm2=_c["imm2"])
                    bs[h] = bh
                _ce_step()
                for it in range(2, SINK_ITR + 1):
                    # u-update: py = KT@b ; a = p1 * recip(py)
                    pys = []
                    for h in range(2):
                        py = psb.tile([128, HB], dt, tag="big",
                                      name=f"py{it}{h}")
                        nc.tensor.matmul(py[:], KT[:], bs[h][:],
                                         start=True, stop=True)
                        pys.append(py)
                    rsl = []
                    for h in range(2):
                        r = scr.tile([128, HB], dtb, tag=f"r{h}",
                                     name=f"r{it}{h}")
                        nc.vector._custom_dve(_RAF, out=r[:], in0=pys[h][:],
                                              s0=_c["s0"], s1=_c["s1"],
                                              imm2=_c["imm2"])
                        rsl.append(r)
                    for h in range(2):
                        a = scr.tile([128, HB], dtb, tag=f"a{h}",
                                     name=f"a{it}{h}")
                        eng = nc.vector if h == 0 else nc.gpsimd
                        eng.tensor_mul(a[:], p1T[:, h * HB:(h + 1) * HB],
                                       rsl[h][:])
                        As[h] = a
                    _ce_step()
                    # v-update: pz = K2@a ; b = recip(pz)
                    pzs = []
                    for h in range(2):
                        pz = psb.tile([128, HB], dt, tag="big",
                                      name=f"pz{it}{h}")
                        nc.tensor.matmul(pz[:], K2[:], As[h][:],
                                         start=True, stop=True)
                        pzs.append(pz)
                    if it == SINK_ITR:
                        for h in range(2):
                            pw = psb.tile([128, HB], dt, tag="big",
                                          name=f"pw{h}")
                            nc.tensor.matmul(pw[:], KC[:], As[h][:],
                                             start=True, stop=True)
                            pws[h] = pw
                    for h in range(2):
                        bh = scr.tile([128, HB], dtb, tag=f"b{h}",
                                      name=f"b{it}{h}")
                        nc.vector._custom_dve(_RAF, out=bh[:], in0=pzs[h][:],
                                              s0=_c["s0"], s1=_c["s1"],
                                              imm2=_c["imm2"])
                        bs[h] = bh
                    _ce_step()
                if stage == 5:
                    dbg = sb.tile([128, 1], dt, tag="dbg5", name="dbg5")
                    nc.vector.tensor_copy(dbg[:], bs[0][:, 0:1])

            if stage >= 6:
                # ---- wcp epilogue: fused mul+reduce ----
                wcp_part = sb.tile([128, 1], dt, tag="wcp_part",
                                   name="wcp_part")
                wp = []
                for h in range(2):
                    scrW = scr.tile([128, HB], dt, tag=f"w{h}",
                                    name=f"scrW{h}")
                    wph = sb.tile([128, 1], dt, tag=f"wcp{h}", name=f"wcp{h}")
                    nc.vector._custom_dve(_TTR, out=scrW[:], in0=pws[h][:],
                                          in1=bs[h][:], s0=0.0, s1=1.0,
                                          imm2=0.0, accum_out=wph[:])
                    wp.append(wph)
                # drain remaining CE ops
                for _ in ce_gen:
                    pass
                ce_part = ce_box[0] if ce_box else None
                nc.vector.tensor_add(wcp_part[:], wp[0][:], wp[1][:])

            # ---------------- pack + store ----------------
            if wcp_part is not None:
                ptO = pst.tile([1, 128], dt, tag="pt", name="ptO")
                nc.tensor.transpose(ptO[:], wcp_part[:], I_f[:])
                nc.vector.tensor_copy(outS[0:1, 0:128], ptO[:])
            elif dbg is not None:
                p = min(dbg.shape[0], 128)
                ptO = pst.tile([1, 128], dt, tag="pt", name="ptO")
                nc.tensor.transpose(ptO[:, 0:p], dbg[0:p, 0:1], I_f[0:p, 0:p])
                nc.vector.tensor_copy(outS[0:1, 0:p], ptO[:, 0:p])
            if ce_part is not None:
                ptC = pst.tile([1, RPC], dt, tag="pt", name="ptC")
                nc.tensor.transpose(ptC[:], ce_part[:], I_f[0:RPC, 0:RPC])
                nc.vector.tensor_copy(outS[0:1, 128:128 + RPC], ptC[:])
            nc.sync.dma_start(out=outd[:], in_=outS[:])

    nc.compile()
    return nc


def _get_nc(stage=99):
    key = ("nc", stage)
    if key not in _CACHE:
        _CACHE[key] = _build_nc(stage)
    return _CACHE[key]


def _make_in_maps(features):
    import ml_dtypes
    featT = np.ascontiguousarray(features.T).astype(ml_dtypes.bfloat16)
    in_maps = []
    for c in range(NCORES):
        off = (c % 2) * 64
        maskce = np.zeros((RPC, B), dtype=ml_dtypes.bfloat16)
        maskce[np.arange(RPC), off + np.arange(RPC)] = 1.0
        in_maps.append({
            "featT": featT,
            "fslice": np.ascontiguousarray(features[c * RPC:(c + 1) * RPC, :]),
            "maskce": maskce,
        })
    return in_maps


def kernel(features, batch=None, **kwargs):
    from concourse.bass_utils import run_bass_kernel_spmd

    features = np.ascontiguousarray(np.asarray(features, dtype=np.float32))
    assert features.shape == (N, D)

    nc = _get_nc()
    res = run_bass_kernel_spmd(nc, _make_in_maps(features),
                               list(range(NCORES)))

    ce_sum = 0.0
    wcp_sum = 0.0
    for c in range(NCORES):
        o = res.results[c]["out"]
        wcp_sum += float(o[0, 0:128].sum(dtype=np.float64))
        ce_sum += float(o[0, 128:128 + RPC].sum(dtype=np.float64))
    loss = ce_sum / M_TOT + wcp_sum / M_TOT
    return np.float32(loss)


if __name__ == "__main__":
    x = np.random.randn(N, D).astype(np.float32)
    print(kernel(x, B))


# revision 16
# speedup vs baseline: 1.1069x; 1.1069x over previous
"""Trainium2 Bass kernel for the CPN/WCP loss (ce + Sinkhorn wcp).

Strategy (v4):
  - Host ships features.T in bf16 ("featT") -> no on-chip F transposes;
    fp32 row slab "fslice" for 4 tiny fsT transposes; bf16 target mask.
  - All 8 featT half-tile DMAs ride the sync+gpsimd queues so the ACT
    queue is free to square FT[q] tiles the moment they land.
  - Per q-chunk the PE interleaves: G0 (Gram), ph dot, ph -0.5*colsum
    (negq x FT[q]^2), and a [128,1] diag-accumulate (gT[q]^2 x ones) so
    every consumer chain starts as early as possible.
  - cost branch: dg = diag(G0) comes from the gT^2 matmuls; rn =
    rsqrt(dg) via Ln/Exp; only the COLUMN normalization (rnB broadcast
    matmul) is applied -- the per-row scale cancels in the row min/max
    normalization. Scalar tail fused with AFFINE_THEN_ADD:
    costm = y*(-G*rden) + (ymax*G*rden) + I in one DVE op.
  - p1 softmax bias-free; CE via per-window max (mh) + masked target
    column (dcol): ce = lnS54 + S5*(mh-dcol), emitted into loop bubbles.
  - Multiplicative Sinkhorn, 2 half-chains of 128 problems, 3 iters
    (converged to <1e-5 of the 5-iter reference by iter 1):
    a = p1*recip(KT@b), b = recip(K2@a), K2 = 128K folds p2; iter-1
    uses a1 = p1*recip(rowsum(K2)) (scale cancels in the bilinear wcp).
  - wcp: pw = KC@a_last; TENSOR_TENSOR_REDUCE fuses pw*b + row-reduce.
"""

import sys

for _p in ("/opt/trn_rl_repo",):
    if _p not in sys.path:
        sys.path.insert(0, _p)

import numpy as np

AUG = 4
B = 128
D = 512
N = AUG * B          # 512 feature rows
NCORES = 8
RPC = N // NCORES    # 64 slab rows per core
MPC = RPC * AUG      # 256 sinkhorn problems per core
HB = MPC // 2        # 128 problems per half-chain
M_TOT = N * AUG      # 2048
TEMP = 5.0
GAMMA = 0.2
SINK_ITR = 3
SCALE1 = 2.0 / float(np.sqrt(np.float32(D)))
SCALE5 = 2.0 / TEMP
LN128 = float(np.log(128.0))

_CACHE = {}


def _build_nc(stage=99):
    import concourse.bacc as bacc
    import concourse.tile as tile
    import concourse.mybir as mybir
    from concourse.dve_ops import (RECIP_APPROX_FAST_CONSTS as _RAFC,
                                   RECIPROCAL_APPROX_FAST as _RAF,
                                   TENSOR_TENSOR_REDUCE as _TTR,
                                   AFFINE_THEN_ADD as _ATA)

    dt = mybir.dt.float32
    dtb = mybir.dt.bfloat16
    fp = mybir.ActivationFunctionType
    alu = mybir.AluOpType
    ax = mybir.AxisListType

    nc = bacc.Bacc(
        "TRN2",
        target_bir_lowering=False,
        debug=False,
        enable_asserts=False,
        num_devices=NCORES,
    )

    featT = nc.dram_tensor("featT", [D, N], dtb, kind="ExternalInput").ap()
    fsl = nc.dram_tensor("fslice", [RPC, D], dt, kind="ExternalInput").ap()
    mce = nc.dram_tensor("maskce", [RPC, B], dtb, kind="ExternalInput").ap()
    outd = nc.dram_tensor("out", [1, 256], dt, kind="ExternalOutput").ap()

    with tile.TileContext(nc) as tc:
        with (
            tc.tile_pool(name="sb", bufs=1) as sb,
            tc.tile_pool(name="scr", bufs=2) as scr,
            tc.tile_pool(name="ps_big", bufs=4, space="PSUM") as psb,
            tc.tile_pool(name="ps_t", bufs=2, space="PSUM") as pst,
            tc.tile_pool(name="ps_h", bufs=1, space="PSUM") as psh,
            tc.tile_pool(name="ps_s", bufs=1, space="PSUM") as pss,
        ):
            dbg = None

            _tabs = list(__import__("concourse.hw_specs",
                                    fromlist=["hw_specs"]
                                    ).get_activation_tables(nc.m.arch))
            _set_lnexp = _tabs.index("natural_log_exp_and_others")
            nc.scalar.add_instruction(mybir.InstLoadActFuncSet(
                name=nc.get_next_instruction_name(), ins=[], outs=[],
                act_func_set_id=_set_lnexp))

            # constants that gate PE (I_f first on gpsimd)
            ones_f = sb.tile([128, 128], dt, tag="ones_f", name="ones_f")
            nc.vector.memset(ones_f[:], 1.0)
            I_f = sb.tile([128, 128], dt, tag="I_f", name="I_f")
            nc.gpsimd.affine_select(I_f[:], ones_f[:], [[1, 128]],
                                    alu.is_equal, 0.0, base=0,
                                    channel_multiplier=-1)

            # ---- DMAs: fs first on sync; FT halves on sync+gpsimd only
            fs = sb.tile([RPC, D], dt, tag="fs", name="fs")
            nc.sync.dma_start(out=fs[:], in_=fsl[:])
            FT = [sb.tile([128, N], dtb, tag=f"FT{q}", name=f"FT{q}")
                  for q in range(4)]
            for q, h, eng in ((0, 0, nc.sync), (0, 1, nc.gpsimd),
                              (1, 0, nc.sync), (1, 1, nc.gpsimd),
                              (2, 0, nc.sync), (2, 1, nc.gpsimd),
                              (3, 0, nc.sync), (3, 1, nc.gpsimd)):
                eng.dma_start(
                    out=FT[q][h * 64:(h + 1) * 64, :],
                    in_=featT[q * 128 + h * 64:q * 128 + (h + 1) * 64, :])

            # remaining constants
            ones_b = sb.tile([128, 128], dtb, tag="ones_b", name="ones_b")
            nc.vector.memset(ones_b[:], 1.0)
            I_b = sb.tile([128, 128], dtb, tag="I_b", name="I_b")
            nc.gpsimd.affine_select(I_b[:], ones_b[:], [[1, 128]],
                                    alu.is_equal, 0.0, base=0,
                                    channel_multiplier=-1)
            negq = sb.tile([128, RPC], dtb, tag="negq", name="negq")
            nc.gpsimd.memset(negq[:], -0.5)
            onecol = sb.tile([128, 1], dtb, tag="onecol", name="onecol")
            nc.gpsimd.memset(onecol[:], 1.0)
            ln128t = sb.tile([128, 1], dt, tag="ln128t", name="ln128t")
            nc.gpsimd.memset(ln128t[:], LN128)
            outS = sb.tile([1, 256], dt, tag="outS", name="outS")
            nc.vector.memset(outS[:], 0.0)

            ce_part = None
            wcp_part = None

            if stage >= 1:
                # fsT via PE transposes (fs + I_f land first)
                fsT = []
                for q in range(4):
                    pt = pst.tile([128, RPC], dt, tag="pt", name=f"ptf{q}")
                    nc.tensor.transpose(pt[:], fs[:, q * 128:(q + 1) * 128],
                                        I_f[:RPC, :RPC])
                    fq = sb.tile([128, RPC], dtb, tag=f"fsT{q}",
                                 name=f"fsT{q}")
                    nc.vector.tensor_copy(fq[:], pt[:])
                    fsT.append(fq)

                # FT^2 on ACT (its queue is otherwise idle until E1)
                FTsq = []
                for q in range(4):
                    sq = scr.tile([128, N], dtb, tag=f"FTsq{q & 1}",
                                  name=f"FTsq{q}")
                    nc.scalar.activation(sq[:], FT[q][:], fp.Square)
                    FTsq.append(sq)

                # gT[q] = sum of the 4 column blocks of FT[q];
                # gsq[q] = gT[q]^2 (gpsimd) feeds the diag matmuls
                gT = []
                gsq = []
                for q in range(4):
                    t1 = scr.tile([128, 128], dtb, tag="gt1", name=f"gt1{q}")
                    nc.vector.tensor_add(t1[:], FT[q][:, 0:128],
                                         FT[q][:, 128:256])
                    t2 = scr.tile([128, 128], dtb, tag="gt2", name=f"gt2{q}")
                    nc.gpsimd.tensor_add(t2[:], FT[q][:, 256:384],
                                         FT[q][:, 384:512])
                    gq = sb.tile([128, 128], dtb, tag=f"gT{q}", name=f"gT{q}")
                    nc.vector.tensor_add(gq[:], t1[:], t2[:])
                    gT.append(gq)
                    gs = scr.tile([128, 128], dtb, tag="gsq", name=f"gsq{q}")
                    nc.gpsimd.tensor_mul(gs[:], gq[:], gq[:])
                    gsq.append(gs)
                dbg = fsT[0]

            if stage >= 2:
                # PE per q: G0 (Gram), ph dot, ph sq, dg diag-accumulate
                ph = psh.tile([RPC, D], dt, tag="ph", name="ph")
                G0 = psb.tile([128, 128], dt, tag="big", name="G0")
                dg = pss.tile([128, 1], dt, tag="dg", name="dg")
                for q in range(4):
                    nc.tensor.matmul(G0[:], gT[q][:], gT[q][:],
                                     start=(q == 0), stop=(q == 3))
                    nc.tensor.matmul(ph[:], fsT[q][:], FT[q][:],
                                     start=(q == 0), stop=False)
                    nc.tensor.matmul(ph[:], negq[:], FTsq[q][:],
                                     start=False, stop=(q == 3))
                    nc.tensor.matmul(dg[:], gsq[q][:], onecol[:, 0:1],
                                     start=(q == 0), stop=(q == 3))
                if stage == 2:
                    dbg = sb.tile([RPC, 1], dt, tag="dbg2", name="dbg2")
                    nc.vector.tensor_copy(dbg[:], ph[:, 0:1])

            if stage >= 3:
                # ---- cost branch ----
                # rn = rsqrt(diag) ; rnB = column-broadcast of rn
                lndg = sb.tile([128, 1], dt, tag="lndg", name="lndg")
                nc.scalar.activation(lndg[:], dg[:], fp.Ln)
                rnb = sb.tile([128, 1], dtb, tag="rnb", name="rnb")
                nc.scalar.activation(rnb[:], lndg[:], fp.Exp, scale=-0.5)
                rnB = psb.tile([128, 128], dt, tag="big", name="rnB")
                nc.tensor.matmul(rnB[:], rnb[:, 0:1].to_broadcast((128, 128)),
                                 I_b[:], start=True, stop=True)
                # y = G0 * rn_j (row scale cancels in row min/max norm);
                # two PSUM reads are illegal -> stage G0 through SBUF
                G0s = sb.tile([128, 128], dt, tag="G0s", name="G0s")
                nc.vector.tensor_copy(G0s[:], G0[:])
                y = sb.tile([128, 128], dt, tag="y", name="y")
                nc.vector.tensor_mul(y[:], G0s[:], rnB[:])
                ymax = sb.tile([128, 1], dt, tag="ymax", name="ymax")
                nc.vector.tensor_reduce(ymax[:], y[:], axis=ax.X, op=alu.max)
                ymin = sb.tile([128, 1], dt, tag="ymin", name="ymin")
                nc.vector.tensor_reduce(ymin[:], y[:], axis=ax.X, op=alu.min)
                # costm = y*(-G*rden) + (ymax*G*rden) + I  (fused)
                den = sb.tile([128, 1], dt, tag="den", name="den")
                nc.vector.tensor_sub(den[:], ymax[:], ymin[:])
                rden = sb.tile([128, 1], dt, tag="rden", name="rden")
                nc.vector.reciprocal(rden[:], den[:])
                sGn = sb.tile([128, 1], dt, tag="sGn", name="sGn")
                nc.vector.tensor_scalar_mul(sGn[:], rden[:], -GAMMA)
                c1t = sb.tile([128, 1], dt, tag="c1t", name="c1t")
                nc.vector.tensor_scalar(
                    out=c1t[:], in0=ymax[:], scalar1=sGn[:, 0:1],
                    scalar2=-1.0, op0=alu.mult, op1=alu.mult)
                costm = sb.tile([128, 128], dt, tag="costm", name="costm")
                nc.vector._custom_dve(_ATA, out=costm[:], in0=y[:],
                                      in1=I_f[:], s0=sGn[:, 0:1],
                                      s1=c1t[:, 0:1], imm2=0.0)

                # ---- p1 path (row layout) ----
                E1 = sb.tile([RPC, D], dt, tag="E1", name="E1")
                nc.scalar.activation(E1[:], ph[:], fp.Exp, scale=SCALE1)
                S14 = sb.tile([RPC, 4], dt, tag="S14", name="S14")
                nc.vector.tensor_reduce(
                    S14[:], E1[:].rearrange("p (k x) -> p k x", k=4),
                    axis=ax.X, op=alu.add)
                rS14 = sb.tile([RPC, 4], dt, tag="rS14", name="rS14")
                nc.vector.reciprocal(rS14[:], S14[:])
                p1r = sb.tile([RPC, D], dt, tag="p1r", name="p1r")
                for k in range(4):
                    ksl = slice(k * 128, (k + 1) * 128)
                    nc.vector.tensor_scalar(
                        out=p1r[:, ksl], in0=E1[:, ksl],
                        scalar1=rS14[:, k:k + 1], scalar2=1e-12,
                        op0=alu.mult, op1=alu.add)

                # ---- K tiles (K2 gates the loop entry) ----
                K2 = sb.tile([128, 128], dtb, tag="K2", name="K2")
                nc.scalar.activation(K2[:], costm[:], fp.Exp,
                                     bias=ln128t[:, 0:1], scale=-2.0)
                ptK = pst.tile([128, 128], dt, tag="pt", name="ptK")
                nc.tensor.transpose(ptK[:], costm[:], I_f[:])
                KT = sb.tile([128, 128], dtb, tag="KT", name="KT")
                nc.scalar.activation(KT[:], ptK[:], fp.Exp, scale=-2.0)
                K = sb.tile([128, 128], dt, tag="K", name="K")
                nc.scalar.activation(K[:], costm[:], fp.Exp, scale=-2.0)
                KC = sb.tile([128, 128], dtb, tag="KC", name="KC")
                nc.gpsimd.tensor_mul(KC[:], K[:], costm[:])
                rsum = sb.tile([128, 1], dt, tag="rsum", name="rsum")
                nc.vector.tensor_reduce(rsum[:], K2[:], axis=ax.X, op=alu.add)
                rs0 = sb.tile([128, 1], dt, tag="rs0", name="rs0")
                nc.vector.reciprocal(rs0[:], rsum[:])
                if stage == 3:
                    dbg = sb.tile([128, 1], dt, tag="dbg3", name="dbg3")
                    nc.vector.tensor_copy(dbg[:], K[:, 0:1])

            if stage >= 4:
                # ---- p1T transposes (copies on ACT) ----
                p1T = sb.tile([128, MPC], dtb, tag="p1T", name="p1T")
                for k in range(4):
                    pt = pst.tile([128, RPC], dt, tag="pt", name=f"ptp{k}")
                    nc.tensor.transpose(pt[:], p1r[:, k * 128:(k + 1) * 128],
                                        I_f[:RPC, :RPC])
                    nc.scalar.copy(p1T[:, k * RPC:(k + 1) * RPC], pt[:])
                if stage == 4:
                    dbg = sb.tile([128, 1], dt, tag="dbg4", name="dbg4")
                    nc.vector.tensor_copy(dbg[:], p1T[:, 0:1])

            # mk only needed mid-loop; keep its DMA late on the ACT queue
            mk = sb.tile([RPC, B], dtb, tag="mk", name="mk")
            nc.scalar.dma_start(out=mk[:], in_=mce[:])

            # CE ops emitted interleaved into the loop below (they only
            # feed the output, and DVE has loop bubbles).
            ce_box = []

            def _ce_ops():
                mh = sb.tile([RPC, 4], dt, tag="mh", name="mh")
                nc.vector.tensor_reduce(
                    mh[:], ph[:].rearrange("p (k x) -> p k x", k=4),
                    axis=ax.X, op=alu.max)
                yield
                dcm = scr.tile([RPC, D], dt, tag="dcm", name="dcm")
                nc.vector.tensor_mul(dcm[:, 0:128], ph[:, 0:128], mk[:])
                nc.vector.tensor_mul(dcm[:, 128:256], ph[:, 128:256], mk[:])
                yield
                bias5 = sb.tile([RPC, 4], dt, tag="bias5", name="bias5")
                nc.gpsimd.tensor_scalar_mul(bias5[:], mh[:], -SCALE5)
                E2 = scr.tile([RPC, D], dt, tag="E2", name="E2")
                for k in range(4):
                    ksl = slice(k * 128, (k + 1) * 128)
                    nc.scalar.activation(E2[:, ksl], ph[:, ksl], fp.Exp,
                                         bias=bias5[:, k:k + 1], scale=SCALE5)
                nc.vector.tensor_mul(dcm[:, 256:384], ph[:, 256:384], mk[:])
                nc.vector.tensor_mul(dcm[:, 384:512], ph[:, 384:512], mk[:])
                yield
                dcol4 = sb.tile([RPC, 4], dt, tag="dcol4", name="dcol4")
                nc.vector.tensor_reduce(
                    dcol4[:], dcm[:].rearrange("p (k x) -> p k x", k=4),
                    axis=ax.X, op=alu.add)
                yield
                S54 = sb.tile([RPC, 4], dt, tag="S54", name="S54")
                nc.vector.tensor_reduce(
                    S54[:], E2[:].rearrange("p (k x) -> p k x", k=4),
                    axis=ax.X, op=alu.add)
                lnS54 = sb.tile([RPC, 4], dt, tag="lnS54", name="lnS54")
                nc.scalar.activation(lnS54[:], S54[:], fp.Ln)
                ce4a = sb.tile([RPC, 4], dt, tag="ce4a", name="ce4a")
                nc.gpsimd.tensor_sub(ce4a[:], mh[:], dcol4[:])
                yield
                ce4 = sb.tile([RPC, 4], dt, tag="ce4", name="ce4")
                nc.vector.scalar_tensor_tensor(
                    out=ce4[:], in0=ce4a[:], scalar=SCALE5,
                    in1=lnS54[:], op0=alu.mult, op1=alu.add)
                cep = sb.tile([RPC, 1], dt, tag="ce_part", name="ce_part")
                nc.vector.tensor_reduce(cep[:], ce4[:], axis=ax.X,
                                        op=alu.add)
                ce_box.append(cep)
                yield

            if stage >= 5:
                ce_gen = _ce_ops()

                def _ce_step():
                    try:
                        next(ce_gen)
                    except StopIteration:
                        pass

                # ---- Sinkhorn loop ----
                _c = _RAFC
                As = [None, None]
                bs = [None, None]
                pws = [None, None]
                # iter 1: a1 = p1 * rs0 (per-class scalar)
                for h in range(2):
                    a = scr.tile([128, HB], dtb, tag=f"a{h}", name=f"a1{h}")
                    nc.vector.tensor_scalar_mul(
                        a[:], p1T[:, h * HB:(h + 1) * HB], rs0[:, 0:1])
                    As[h] = a
                pzs = []
                for h in range(2):
                    pz = psb.tile([128, HB], dt, tag="big", name=f"pz1{h}")
                    nc.tensor.matmul(pz[:], K2[:], As[h][:],
                                     start=True, stop=True)
                    pzs.append(pz)
                for h in range(2):
                    bh = scr.tile([128, HB], dtb, tag=f"b{h}", name=f"b1{h}")
                    nc.vector._custom_dve(_RAF, out=bh[:], in0=pzs[h][:],
                                          s0=_c["s0"], s1=_c["s1"],
                                          imm2=_c["imm2"])
                    bs[h] = bh
                _ce_step()
                for it in range(2, SINK_ITR + 1):
                    # u-update: py = KT@b ; a = p1 * recip(py)
                    pys = []
                    for h in range(2):
                        py = psb.tile([128, HB], dt, tag="big",
                                      name=f"py{it}{h}")
                        nc.tensor.matmul(py[:], KT[:], bs[h][:],
                                         start=True, stop=True)
                        pys.append(py)
                    rsl = []
                    for h in range(2):
                        r = scr.tile([128, HB], dtb, tag=f"r{h}",
                                     name=f"r{it}{h}")
                        nc.vector._custom_dve(_RAF, out=r[:], in0=pys[h][:],
                                              s0=_c["s0"], s1=_c["s1"],
                                              imm2=_c["imm2"])
                        rsl.append(r)
                    for h in range(2):
                        a = scr.tile([128, HB], dtb, tag=f"a{h}",
                                     name=f"a{it}{h}")
                        eng = nc.vector if h == 0 else nc.gpsimd
                        eng.tensor_mul(a[:], p1T[:, h * HB:(h + 1) * HB],
                                       rsl[h][:])
                        As[h] = a
                    _ce_step()
                    # v-update: pz = K2@a ; b = recip(pz)
                    pzs = []
                    for h in range(2):
                        pz = psb.tile([128, HB], dt, tag="big",
                                      name=f"pz{it}{h}")
                        nc.tensor.matmul(pz[:], K2[:], As[h][:],
                                         start=True, stop=True)
                        pzs.append(pz)
                    if it == SINK_ITR:
                        for h in range(2):
                            pw = psb.tile([128, HB], dt, tag="big",
                                          name=f"pw{h}")
                            nc.tensor.matmul(pw[:], KC[:], As[h][:],
                                             start=True, stop=True)
                            pws[h] = pw
                    for h in range(2):
                        bh = scr.tile([128, HB], dtb, tag=f"b{h}",
                                      name=f"b{it}{h}")
                        nc.vector._custom_dve(_RAF, out=bh[:], in0=pzs[h][:],
                                              s0=_c["s0"], s1=_c["s1"],
                                              imm2=_c["imm2"])
                        bs[h] = bh
                    _ce_step()
                if stage == 5:
                    dbg = sb.tile([128, 1], dt, tag="dbg5", name="dbg5")
                    nc.vector.tensor_copy(dbg[:], bs[0][:, 0:1])

            if stage >= 6:
                # ---- wcp epilogue: fused mul+reduce ----
                wcp_part = sb.tile([128, 1], dt, tag="wcp_part",
                                   name="wcp_part")
                wp = []
                for h in range(2):
                    scrW = scr.tile([128, HB], dt, tag=f"w{h}",
                                    name=f"scrW{h}")
                    wph = sb.tile([128, 1], dt, tag=f"wcp{h}", name=f"wcp{h}")
                    nc.vector._custom_dve(_TTR, out=scrW[:], in0=pws[h][:],
                                          in1=bs[h][:], s0=0.0, s1=1.0,
                                          imm2=0.0, accum_out=wph[:])
                    wp.append(wph)
                # drain remaining CE ops
                for _ in ce_gen:
                    pass
                ce_part = ce_box[0] if ce_box else None
                nc.vector.tensor_add(wcp_part[:], wp[0][:], wp[1][:])

            # ---------------- pack + store ----------------
            if wcp_part is not None:
                ptO = pst.tile([1, 128], dt, tag="pt", name="ptO")
                nc.tensor.transpose(ptO[:], wcp_part[:], I_f[:])
                nc.vector.tensor_copy(outS[0:1, 0:128], ptO[:])
            elif dbg is not None:
                p = min(dbg.shape[0], 128)
                ptO = pst.tile([1, 128], dt, tag="pt", name="ptO")
                nc.tensor.transpose(ptO[:, 0:p], dbg[0:p, 0:1], I_f[0:p, 0:p])
                nc.vector.tensor_copy(outS[0:1, 0:p], ptO[:, 0:p])
            if ce_part is not None:
                ptC = pst.tile([1, RPC], dt, tag="pt", name="ptC")
                nc.tensor.transpose(ptC[:], ce_part[:], I_f[0:RPC, 0:RPC])
                nc.vector.tensor_copy(outS[0:1, 128:128 + RPC], ptC[:])
            nc.sync.dma_start(out=outd[:], in_=outS[:])

    nc.compile()
    return nc


def _get_nc(stage=99):
    key = ("nc", stage)
    if key not in _CACHE:
        _CACHE[key] = _build_nc(stage)
    return _CACHE[key]


def _make_in_maps(features):
    import ml_dtypes
    featT = np.ascontiguousarray(features.T).astype(ml_dtypes.bfloat16)
    in_maps = []
    for c in range(NCORES):
        off = (c % 2) * 64
        maskce = np.zeros((RPC, B), dtype=ml_dtypes.bfloat16)
        maskce[np.arange(RPC), off + np.arange(RPC)] = 1.0
        in_maps.append({
            "featT": featT,
            "fslice": np.ascontiguousarray(features[c * RPC:(c + 1) * RPC, :]),
            "maskce": maskce,
        })
    return in_maps


def kernel(features, batch=None, **kwargs):
    from concourse.bass_utils import run_bass_kernel_spmd

    features = np.ascontiguousarray(np.asarray(features, dtype=np.float32))
    assert features.shape == (N, D)

    nc = _get_nc()
    res = run_bass_kernel_spmd(nc, _make_in_maps(features),
                               list(range(NCORES)))

    ce_sum = 0.0
    wcp_sum = 0.0
    for c in range(NCORES):
        o = res.results[c]["out"]
        wcp_sum += float(o[0, 0:128].sum(dtype=np.float64))
        ce_sum += float(o[0, 128:128 + RPC].sum(dtype=np.float64))
    loss = ce_sum / M_TOT + wcp_sum / M_TOT
    return np.float32(loss)


if __name__ == "__main__":
    x = np.random.randn(N, D).astype(np.float32)
    print(kernel(x, B))


# revision 17
# speedup vs baseline: 1.1429x; 1.0325x over previous
"""Trainium2 Bass kernel for the CPN/WCP loss (ce + Sinkhorn wcp).

Strategy (v4):
  - Host ships features.T in bf16 ("featT") -> no on-chip F transposes;
    fp32 row slab "fslice" for 4 tiny fsT transposes; bf16 target mask.
  - All 8 featT half-tile DMAs ride the sync+gpsimd queues so the ACT
    queue is free to square FT[q] tiles the moment they land.
  - Per q-chunk the PE interleaves: G0 (Gram), ph dot, ph -0.5*colsum
    (negq x FT[q]^2), and a [128,1] diag-accumulate (gT[q]^2 x ones) so
    every consumer chain starts as early as possible.
  - cost branch: dg = diag(G0) comes from the gT^2 matmuls; rn =
    rsqrt(dg) via Ln/Exp; only the COLUMN normalization (rnB broadcast
    matmul) is applied -- the per-row scale cancels in the row min/max
    normalization. Scalar tail fused with AFFINE_THEN_ADD:
    costm = y*(-G*rden) + (ymax*G*rden) + I in one DVE op.
  - p1 softmax bias-free; CE via per-window max (mh) + masked target
    column (dcol): ce = lnS54 + S5*(mh-dcol), emitted into loop bubbles.
  - Multiplicative Sinkhorn, 2 half-chains of 128 problems, 3 iters
    (converged to <1e-5 of the 5-iter reference by iter 1):
    a = p1*recip(KT@b), b = recip(K2@a), K2 = 128K folds p2; iter-1
    uses a1 = p1*recip(rowsum(K2)) (scale cancels in the bilinear wcp).
  - wcp: pw = KC@a_last; TENSOR_TENSOR_REDUCE fuses pw*b + row-reduce.
"""

import sys

for _p in ("/opt/trn_rl_repo",):
    if _p not in sys.path:
        sys.path.insert(0, _p)

import numpy as np

AUG = 4
B = 128
D = 512
N = AUG * B          # 512 feature rows
NCORES = 8
RPC = N // NCORES    # 64 slab rows per core
MPC = RPC * AUG      # 256 sinkhorn problems per core
HB = MPC // 2        # 128 problems per half-chain
M_TOT = N * AUG      # 2048
TEMP = 5.0
GAMMA = 0.2
SINK_ITR = 3
SCALE1 = 2.0 / float(np.sqrt(np.float32(D)))
SCALE5 = 2.0 / TEMP
LN128 = float(np.log(128.0))

_CACHE = {}


def _build_nc(stage=99):
    import concourse.bacc as bacc
    import concourse.tile as tile
    import concourse.mybir as mybir
    from concourse.dve_ops import (RECIP_APPROX_FAST_CONSTS as _RAFC,
                                   RECIPROCAL_APPROX_FAST as _RAF,
                                   TENSOR_TENSOR_REDUCE as _TTR,
                                   AFFINE_THEN_ADD as _ATA)

    dt = mybir.dt.float32
    dtb = mybir.dt.bfloat16
    fp = mybir.ActivationFunctionType
    alu = mybir.AluOpType
    ax = mybir.AxisListType

    nc = bacc.Bacc(
        "TRN2",
        target_bir_lowering=False,
        debug=False,
        enable_asserts=False,
        num_devices=NCORES,
    )

    featT = nc.dram_tensor("featT", [D, N], dtb, kind="ExternalInput").ap()
    fsl = nc.dram_tensor("fslice", [RPC, D], dt, kind="ExternalInput").ap()
    mce = nc.dram_tensor("maskce", [RPC, B], dtb, kind="ExternalInput").ap()
    outd = nc.dram_tensor("out", [1, 256], dt, kind="ExternalOutput").ap()

    with tile.TileContext(nc) as tc:
        with (
            tc.tile_pool(name="sb", bufs=1) as sb,
            tc.tile_pool(name="scr", bufs=2) as scr,
            tc.tile_pool(name="ps_big", bufs=4, space="PSUM") as psb,
            tc.tile_pool(name="ps_t", bufs=2, space="PSUM") as pst,
            tc.tile_pool(name="ps_h", bufs=1, space="PSUM") as psh,
            tc.tile_pool(name="ps_s", bufs=1, space="PSUM") as pss,
        ):
            dbg = None

            _tabs = list(__import__("concourse.hw_specs",
                                    fromlist=["hw_specs"]
                                    ).get_activation_tables(nc.m.arch))
            _set_lnexp = _tabs.index("natural_log_exp_and_others")
            nc.scalar.add_instruction(mybir.InstLoadActFuncSet(
                name=nc.get_next_instruction_name(), ins=[], outs=[],
                act_func_set_id=_set_lnexp))

            # constants that gate PE (I_f first on gpsimd)
            ones_f = sb.tile([128, 128], dt, tag="ones_f", name="ones_f")
            nc.vector.memset(ones_f[:], 1.0)
            I_f = sb.tile([128, 128], dt, tag="I_f", name="I_f")
            nc.gpsimd.affine_select(I_f[:], ones_f[:], [[1, 128]],
                                    alu.is_equal, 0.0, base=0,
                                    channel_multiplier=-1)

            # ---- DMAs: fs first on sync; FT halves on sync+gpsimd only
            fs = sb.tile([RPC, D], dt, tag="fs", name="fs")
            nc.sync.dma_start(out=fs[:], in_=fsl[:])
            FT = [sb.tile([128, N], dtb, tag=f"FT{q}", name=f"FT{q}")
                  for q in range(4)]
            for q, h, eng in ((0, 0, nc.sync), (0, 1, nc.gpsimd),
                              (1, 0, nc.sync), (1, 1, nc.gpsimd),
                              (2, 0, nc.sync), (2, 1, nc.gpsimd),
                              (3, 0, nc.sync), (3, 1, nc.scalar)):
                eng.dma_start(
                    out=FT[q][h * 64:(h + 1) * 64, :],
                    in_=featT[q * 128 + h * 64:q * 128 + (h + 1) * 64, :])

            # remaining constants
            ones_b = sb.tile([128, 128], dtb, tag="ones_b", name="ones_b")
            nc.vector.memset(ones_b[:], 1.0)
            I_b = sb.tile([128, 128], dtb, tag="I_b", name="I_b")
            nc.gpsimd.affine_select(I_b[:], ones_b[:], [[1, 128]],
                                    alu.is_equal, 0.0, base=0,
                                    channel_multiplier=-1)
            negq = sb.tile([128, RPC], dtb, tag="negq", name="negq")
            nc.gpsimd.memset(negq[:], -0.5)
            onecol = sb.tile([128, 1], dtb, tag="onecol", name="onecol")
            nc.gpsimd.memset(onecol[:], 1.0)
            ln128t = sb.tile([128, 1], dt, tag="ln128t", name="ln128t")
            nc.gpsimd.memset(ln128t[:], LN128)
            outS = sb.tile([1, 256], dt, tag="outS", name="outS")
            nc.vector.memset(outS[:], 0.0)

            ce_part = None
            wcp_part = None

            if stage >= 1:
                # fsT via PE transposes (fs + I_f land first)
                fsT = []
                for q in range(4):
                    pt = pst.tile([128, RPC], dt, tag="pt", name=f"ptf{q}")
                    nc.tensor.transpose(pt[:], fs[:, q * 128:(q + 1) * 128],
                                        I_f[:RPC, :RPC])
                    fq = sb.tile([128, RPC], dtb, tag=f"fsT{q}",
                                 name=f"fsT{q}")
                    nc.vector.tensor_copy(fq[:], pt[:])
                    fsT.append(fq)

                # FT^2 on ACT (its queue is otherwise idle until E1)
                FTsq = []
                for q in range(4):
                    sq = scr.tile([128, N], dtb, tag=f"FTsq{q & 1}",
                                  name=f"FTsq{q}")
                    nc.scalar.activation(sq[:], FT[q][:], fp.Square)
                    FTsq.append(sq)

                # gT[q] = sum of the 4 column blocks of FT[q];
                # gsq[q] = gT[q]^2 (gpsimd) feeds the diag matmuls;
                # sqS = sum_q FTsq[q] feeds ONE ph colsum matmul
                gT = []
                gsq = []
                s01 = scr.tile([128, N], dtb, tag="s01", name="s01")
                s23 = scr.tile([128, N], dtb, tag="s23", name="s23")
                sqS = sb.tile([128, N], dtb, tag="sqS", name="sqS")
                for q in range(4):
                    t1 = scr.tile([128, 128], dtb, tag="gt1", name=f"gt1{q}")
                    nc.vector.tensor_add(t1[:], FT[q][:, 0:128],
                                         FT[q][:, 128:256])
                    t2 = scr.tile([128, 128], dtb, tag="gt2", name=f"gt2{q}")
                    nc.gpsimd.tensor_add(t2[:], FT[q][:, 256:384],
                                         FT[q][:, 384:512])
                    gq = sb.tile([128, 128], dtb, tag=f"gT{q}", name=f"gT{q}")
                    nc.vector.tensor_add(gq[:], t1[:], t2[:])
                    gT.append(gq)
                    gs = scr.tile([128, 128], dtb, tag="gsq", name=f"gsq{q}")
                    nc.gpsimd.tensor_mul(gs[:], gq[:], gq[:])
                    gsq.append(gs)
                    if q == 1:
                        nc.vector.tensor_add(s01[:], FTsq[0][:], FTsq[1][:])
                    elif q == 3:
                        nc.vector.tensor_add(s23[:], FTsq[2][:], FTsq[3][:])
                        nc.vector.tensor_add(sqS[:], s01[:], s23[:])
                dbg = fsT[0]

            if stage >= 2:
                # PE per q: G0 (Gram), dg diag-accumulate, ph dot;
                # one sqS colsum matmul closes ph
                ph = psh.tile([RPC, D], dt, tag="ph", name="ph")
                G0 = psb.tile([128, 128], dt, tag="big", name="G0")
                dg = pss.tile([128, 1], dt, tag="dg", name="dg")
                for q in range(4):
                    nc.tensor.matmul(G0[:], gT[q][:], gT[q][:],
                                     start=(q == 0), stop=(q == 3))
                    nc.tensor.matmul(dg[:], gsq[q][:], onecol[:, 0:1],
                                     start=(q == 0), stop=(q == 3))
                    nc.tensor.matmul(ph[:], fsT[q][:], FT[q][:],
                                     start=(q == 0), stop=False)
                nc.tensor.matmul(ph[:], negq[:], sqS[:],
                                 start=False, stop=True)
                if stage == 2:
                    dbg = sb.tile([RPC, 1], dt, tag="dbg2", name="dbg2")
                    nc.vector.tensor_copy(dbg[:], ph[:, 0:1])

            if stage >= 3:
                # ---- cost branch ----
                # rn = rsqrt(diag) ; rnB = column-broadcast of rn
                lndg = sb.tile([128, 1], dt, tag="lndg", name="lndg")
                nc.scalar.activation(lndg[:], dg[:], fp.Ln)
                rnb = sb.tile([128, 1], dtb, tag="rnb", name="rnb")
                nc.scalar.activation(rnb[:], lndg[:], fp.Exp, scale=-0.5)
                rnB = psb.tile([128, 128], dt, tag="big", name="rnB")
                nc.tensor.matmul(rnB[:], rnb[:, 0:1].to_broadcast((128, 128)),
                                 I_b[:], start=True, stop=True)
                # y = G0 * rn_j (row scale cancels in row min/max norm);
                # two PSUM reads are illegal -> stage G0 through SBUF
                G0s = sb.tile([128, 128], dt, tag="G0s", name="G0s")
                nc.vector.tensor_copy(G0s[:], G0[:])
                y = sb.tile([128, 128], dt, tag="y", name="y")
                nc.vector.tensor_mul(y[:], G0s[:], rnB[:])
                ymax = sb.tile([128, 1], dt, tag="ymax", name="ymax")
                nc.vector.tensor_reduce(ymax[:], y[:], axis=ax.X, op=alu.max)
                ymin = sb.tile([128, 1], dt, tag="ymin", name="ymin")
                nc.vector.tensor_reduce(ymin[:], y[:], axis=ax.X, op=alu.min)
                # costm = y*(-G*rden) + (ymax*G*rden) + I  (fused)
                den = sb.tile([128, 1], dt, tag="den", name="den")
                nc.vector.tensor_sub(den[:], ymax[:], ymin[:])
                rden = sb.tile([128, 1], dt, tag="rden", name="rden")
                nc.vector.reciprocal(rden[:], den[:])
                sGn = sb.tile([128, 1], dt, tag="sGn", name="sGn")
                nc.vector.tensor_scalar_mul(sGn[:], rden[:], -GAMMA)
                c1t = sb.tile([128, 1], dt, tag="c1t", name="c1t")
                nc.vector.tensor_scalar(
                    out=c1t[:], in0=ymax[:], scalar1=sGn[:, 0:1],
                    scalar2=-1.0, op0=alu.mult, op1=alu.mult)
                costm = sb.tile([128, 128], dt, tag="costm", name="costm")
                nc.vector._custom_dve(_ATA, out=costm[:], in0=y[:],
                                      in1=I_f[:], s0=sGn[:, 0:1],
                                      s1=c1t[:, 0:1], imm2=0.0)

                # ---- p1 path (row layout) ----
                E1 = sb.tile([RPC, D], dt, tag="E1", name="E1")
                nc.scalar.activation(E1[:], ph[:], fp.Exp, scale=SCALE1)
                S14 = sb.tile([RPC, 4], dt, tag="S14", name="S14")
                nc.vector.tensor_reduce(
                    S14[:], E1[:].rearrange("p (k x) -> p k x", k=4),
                    axis=ax.X, op=alu.add)
                rS14 = sb.tile([RPC, 4], dt, tag="rS14", name="rS14")
                nc.vector.reciprocal(rS14[:], S14[:])
                p1r = sb.tile([RPC, D], dt, tag="p1r", name="p1r")
                for k in range(4):
                    ksl = slice(k * 128, (k + 1) * 128)
                    nc.vector.tensor_scalar(
                        out=p1r[:, ksl], in0=E1[:, ksl],
                        scalar1=rS14[:, k:k + 1], scalar2=1e-12,
                        op0=alu.mult, op1=alu.add)

                # ---- K tiles (K2 gates the loop entry) ----
                K2 = sb.tile([128, 128], dtb, tag="K2", name="K2")
                nc.scalar.activation(K2[:], costm[:], fp.Exp,
                                     bias=ln128t[:, 0:1], scale=-2.0)
                ptK = pst.tile([128, 128], dt, tag="pt", name="ptK")
                nc.tensor.transpose(ptK[:], costm[:], I_f[:])
                KT = sb.tile([128, 128], dtb, tag="KT", name="KT")
                nc.scalar.activation(KT[:], ptK[:], fp.Exp, scale=-2.0)
                K = sb.tile([128, 128], dt, tag="K", name="K")
                nc.scalar.activation(K[:], costm[:], fp.Exp, scale=-2.0)
                KC = sb.tile([128, 128], dtb, tag="KC", name="KC")
                nc.gpsimd.tensor_mul(KC[:], K[:], costm[:])
                rsum = sb.tile([128, 1], dt, tag="rsum", name="rsum")
                nc.vector.tensor_reduce(rsum[:], K2[:], axis=ax.X, op=alu.add)
                rs0 = sb.tile([128, 1], dt, tag="rs0", name="rs0")
                nc.vector.reciprocal(rs0[:], rsum[:])
                if stage == 3:
                    dbg = sb.tile([128, 1], dt, tag="dbg3", name="dbg3")
                    nc.vector.tensor_copy(dbg[:], K[:, 0:1])

            if stage >= 4:
                # ---- p1T transposes (copies on ACT) ----
                p1T = sb.tile([128, MPC], dtb, tag="p1T", name="p1T")
                for k in range(4):
                    pt = pst.tile([128, RPC], dt, tag="pt", name=f"ptp{k}")
                    nc.tensor.transpose(pt[:], p1r[:, k * 128:(k + 1) * 128],
                                        I_f[:RPC, :RPC])
                    nc.scalar.copy(p1T[:, k * RPC:(k + 1) * RPC], pt[:])
                if stage == 4:
                    dbg = sb.tile([128, 1], dt, tag="dbg4", name="dbg4")
                    nc.vector.tensor_copy(dbg[:], p1T[:, 0:1])

            # mk only needed mid-loop; keep its DMA late on the ACT queue
            mk = sb.tile([RPC, B], dtb, tag="mk", name="mk")
            nc.scalar.dma_start(out=mk[:], in_=mce[:])

            # CE ops emitted interleaved into the loop below (they only
            # feed the output, and DVE has loop bubbles).
            ce_box = []

            def _ce_ops():
                mh = sb.tile([RPC, 4], dt, tag="mh", name="mh")
                nc.vector.tensor_reduce(
                    mh[:], ph[:].rearrange("p (k x) -> p k x", k=4),
                    axis=ax.X, op=alu.max)
                yield
                dcm = scr.tile([RPC, D], dt, tag="dcm", name="dcm")
                nc.vector.tensor_mul(dcm[:, 0:128], ph[:, 0:128], mk[:])
                nc.vector.tensor_mul(dcm[:, 128:256], ph[:, 128:256], mk[:])
                yield
                bias5 = sb.tile([RPC, 4], dt, tag="bias5", name="bias5")
                nc.gpsimd.tensor_scalar_mul(bias5[:], mh[:], -SCALE5)
                E2 = scr.tile([RPC, D], dt, tag="E2", name="E2")
                for k in range(4):
                    ksl = slice(k * 128, (k + 1) * 128)
                    nc.scalar.activation(E2[:, ksl], ph[:, ksl], fp.Exp,
                                         bias=bias5[:, k:k + 1], scale=SCALE5)
                nc.vector.tensor_mul(dcm[:, 256:384], ph[:, 256:384], mk[:])
                nc.vector.tensor_mul(dcm[:, 384:512], ph[:, 384:512], mk[:])
                yield
                dcol4 = sb.tile([RPC, 4], dt, tag="dcol4", name="dcol4")
                nc.vector.tensor_reduce(
                    dcol4[:], dcm[:].rearrange("p (k x) -> p k x", k=4),
                    axis=ax.X, op=alu.add)
                yield
                S54 = sb.tile([RPC, 4], dt, tag="S54", name="S54")
                nc.vector.tensor_reduce(
                    S54[:], E2[:].rearrange("p (k x) -> p k x", k=4),
                    axis=ax.X, op=alu.add)
                lnS54 = sb.tile([RPC, 4], dt, tag="lnS54", name="lnS54")
                nc.scalar.activation(lnS54[:], S54[:], fp.Ln)
                ce4a = sb.tile([RPC, 4], dt, tag="ce4a", name="ce4a")
                nc.gpsimd.tensor_sub(ce4a[:], mh[:], dcol4[:])
                yield
                ce4 = sb.tile([RPC, 4], dt, tag="ce4", name="ce4")
                nc.vector.scalar_tensor_tensor(
                    out=ce4[:], in0=ce4a[:], scalar=SCALE5,
                    in1=lnS54[:], op0=alu.mult, op1=alu.add)
                cep = sb.tile([RPC, 1], dt, tag="ce_part", name="ce_part")
                nc.vector.tensor_reduce(cep[:], ce4[:], axis=ax.X,
                                        op=alu.add)
                ce_box.append(cep)
                yield

            if stage >= 5:
                ce_gen = _ce_ops()

                def _ce_step():
                    try:
                        next(ce_gen)
                    except StopIteration:
                        pass

                # ---- Sinkhorn loop ----
                _c = _RAFC
                As = [None, None]
                bs = [None, None]
                pws = [None, None]
                # iter 1: a1 = p1 * rs0 (per-class scalar)
                for h in range(2):
                    a = scr.tile([128, HB], dtb, tag=f"a{h}", name=f"a1{h}")
                    nc.vector.tensor_scalar_mul(
                        a[:], p1T[:, h * HB:(h + 1) * HB], rs0[:, 0:1])
                    As[h] = a
                pzs = []
                for h in range(2):
                    pz = psb.tile([128, HB], dt, tag="big", name=f"pz1{h}")
                    nc.tensor.matmul(pz[:], K2[:], As[h][:],
                                     start=True, stop=True)
                    pzs.append(pz)
                for h in range(2):
                    bh = scr.tile([128, HB], dtb, tag=f"b{h}", name=f"b1{h}")
                    nc.vector._custom_dve(_RAF, out=bh[:], in0=pzs[h][:],
                                          s0=_c["s0"], s1=_c["s1"],
                                          imm2=_c["imm2"])
                    bs[h] = bh
                _ce_step()
                for it in range(2, SINK_ITR + 1):
                    # u-update: py = KT@b ; a = p1 * recip(py)
                    pys = []
                    for h in range(2):
                        py = psb.tile([128, HB], dt, tag="big",
                                      name=f"py{it}{h}")
                        nc.tensor.matmul(py[:], KT[:], bs[h][:],
                                         start=True, stop=True)
                        pys.append(py)
                    rsl = []
                    for h in range(2):
                        r = scr.tile([128, HB], dtb, tag=f"r{h}",
                                     name=f"r{it}{h}")
                        nc.vector._custom_dve(_RAF, out=r[:], in0=pys[h][:],
                                              s0=_c["s0"], s1=_c["s1"],
                                              imm2=_c["imm2"])
                        rsl.append(r)
                    for h in range(2):
                        a = scr.tile([128, HB], dtb, tag=f"a{h}",
                                     name=f"a{it}{h}")
                        eng = nc.vector if h == 0 else nc.gpsimd
                        eng.tensor_mul(a[:], p1T[:, h * HB:(h + 1) * HB],
                                       rsl[h][:])
                        As[h] = a
                    _ce_step()
                    # v-update: pz = K2@a ; b = recip(pz)
                    pzs = []
                    for h in range(2):
                        pz = psb.tile([128, HB], dt, tag="big",
                                      name=f"pz{it}{h}")
                        nc.tensor.matmul(pz[:], K2[:], As[h][:],
                                         start=True, stop=True)
                        pzs.append(pz)
                    if it == SINK_ITR:
                        for h in range(2):
                            pw = psb.tile([128, HB], dt, tag="big",
                                          name=f"pw{h}")
                            nc.tensor.matmul(pw[:], KC[:], As[h][:],
                                             start=True, stop=True)
                            pws[h] = pw
                    for h in range(2):
                        bh = scr.tile([128, HB], dtb, tag=f"b{h}",
                                      name=f"b{it}{h}")
                        nc.vector._custom_dve(_RAF, out=bh[:], in0=pzs[h][:],
                                              s0=_c["s0"], s1=_c["s1"],
                                              imm2=_c["imm2"])
                        bs[h] = bh
                    _ce_step()
                if stage == 5:
                    dbg = sb.tile([128, 1], dt, tag="dbg5", name="dbg5")
                    nc.vector.tensor_copy(dbg[:], bs[0][:, 0:1])

            if stage >= 6:
                # ---- wcp epilogue: fused mul+reduce ----
                wcp_part = sb.tile([128, 1], dt, tag="wcp_part",
                                   name="wcp_part")
                wp = []
                for h in range(2):
                    scrW = scr.tile([128, HB], dt, tag=f"w{h}",
                                    name=f"scrW{h}")
                    wph = sb.tile([128, 1], dt, tag=f"wcp{h}", name=f"wcp{h}")
                    nc.vector._custom_dve(_TTR, out=scrW[:], in0=pws[h][:],
                                          in1=bs[h][:], s0=0.0, s1=1.0,
                                          imm2=0.0, accum_out=wph[:])
                    wp.append(wph)
                # drain remaining CE ops
                for _ in ce_gen:
                    pass
                ce_part = ce_box[0] if ce_box else None
                nc.vector.tensor_add(wcp_part[:], wp[0][:], wp[1][:])

            # ---------------- pack + store ----------------
            if wcp_part is not None:
                ptO = pst.tile([1, 128], dt, tag="pt", name="ptO")
                nc.tensor.transpose(ptO[:], wcp_part[:], I_f[:])
                nc.vector.tensor_copy(outS[0:1, 0:128], ptO[:])
            elif dbg is not None:
                p = min(dbg.shape[0], 128)
                ptO = pst.tile([1, 128], dt, tag="pt", name="ptO")
                nc.tensor.transpose(ptO[:, 0:p], dbg[0:p, 0:1], I_f[0:p, 0:p])
                nc.vector.tensor_copy(outS[0:1, 0:p], ptO[:, 0:p])
            if ce_part is not None:
                ptC = pst.tile([1, RPC], dt, tag="pt", name="ptC")
                nc.tensor.transpose(ptC[:], ce_part[:], I_f[0:RPC, 0:RPC])
                nc.vector.tensor_copy(outS[0:1, 128:128 + RPC], ptC[:])
            nc.sync.dma_start(out=outd[:], in_=outS[:])

    nc.compile()
    return nc


def _get_nc(stage=99):
    key = ("nc", stage)
    if key not in _CACHE:
        _CACHE[key] = _build_nc(stage)
    return _CACHE[key]


def _make_in_maps(features):
    import ml_dtypes
    featT = np.ascontiguousarray(features.T).astype(ml_dtypes.bfloat16)
    in_maps = []
    for c in range(NCORES):
        off = (c % 2) * 64
        maskce = np.zeros((RPC, B), dtype=ml_dtypes.bfloat16)
        maskce[np.arange(RPC), off + np.arange(RPC)] = 1.0
        in_maps.append({
            "featT": featT,
            "fslice": np.ascontiguousarray(features[c * RPC:(c + 1) * RPC, :]),
            "maskce": maskce,
        })
    return in_maps


def kernel(features, batch=None, **kwargs):
    from concourse.bass_utils import run_bass_kernel_spmd

    features = np.ascontiguousarray(np.asarray(features, dtype=np.float32))
    assert features.shape == (N, D)

    nc = _get_nc()
    res = run_bass_kernel_spmd(nc, _make_in_maps(features),
                               list(range(NCORES)))

    ce_sum = 0.0
    wcp_sum = 0.0
    for c in range(NCORES):
        o = res.results[c]["out"]
        wcp_sum += float(o[0, 0:128].sum(dtype=np.float64))
        ce_sum += float(o[0, 128:128 + RPC].sum(dtype=np.float64))
    loss = ce_sum / M_TOT + wcp_sum / M_TOT
    return np.float32(loss)


if __name__ == "__main__":
    x = np.random.randn(N, D).astype(np.float32)
    print(kernel(x, B))
